# revision 1
# baseline (speedup 1.0000x reference)
"""Trainium2 Bass kernel for the LoRA-update contraction.

Computes out[b,n] = sum_l <B_l @ A_l, gradient[l,b,n]>_F for
  lora_A    [48, 8, 1024]       (L, R, IN)
  lora_B    [48, 1024, 8]       (L, OUT, R)
  gradient  [48, 4, 2, 1024, 1024]  (L, B, N, OUT, IN)

Strategy (memory-bound problem — gradient is 1.6 GB):
  - Shard L across the 8 NeuronCores (6 layers each, 201 MB of gradient per
    core). Per-core partial outputs [B*N] are summed on the host.
  - On each core: W_l = B_l @ A_l is computed once per layer on the
    TensorEngine (fp32, cheap: 50M MACs), then the gradient streams through
    SBUF in 4 MB blocks and a fused VectorEngine tensor_tensor_reduce does
    acc[p] += sum_i G[p,i]*W[p,i] in a single pass at full fp32 precision.
  - A final ones-vector matmul reduces the 128 partition accumulators.
"""

import numpy as np

L, R, OUT, IN = 48, 8, 1024, 1024
B, N = 4, 2
NCORES = 8
LP = L // NCORES  # layers per core
BN = B * N

_PART = 128


def build_module(lp=LP, bn=BN, out_dim=OUT, in_dim=IN, r=R):
    """Build + compile the per-core Bass module (same program on all cores)."""
    import concourse.bacc as bacc
    import concourse.mybir as mybir
    from concourse.tile import TileContext

    fp32 = mybir.dt.float32
    oc = out_dim // _PART          # number of 128-row chunks of OUT
    n_mm = min(512, in_dim)        # matmul moving-dim tile (one PSUM bank)
    ih = in_dim // n_mm
    # TTR chunk: cover `cw` o-chunks per op to amortize DVE op overhead
    cw = 2 if oc % 2 == 0 else 1
    nh = oc // cw

    nc = bacc.Bacc("TRN2", target_bir_lowering=False, debug=False)

    nchunk = lp * (oc // cw)
    g = nc.dram_tensor("g", [lp, bn, out_dim, in_dim], fp32, kind="ExternalInput").ap()
    bt = nc.dram_tensor("bt", [lp, r, out_dim], fp32, kind="ExternalInput").ap()
    a = nc.dram_tensor("a", [lp, r, in_dim], fp32, kind="ExternalInput").ap()
    # Per-(partition, bn, chunk) partial sums; the final reduction over
    # partitions/chunks (a few KB) happens on the host.
    out = nc.dram_tensor("out", [_PART, bn, nchunk], fp32, kind="ExternalOutput").ap()

    with TileContext(nc) as tc:
        with (
            tc.tile_pool(name="gpool", bufs=3) as gpool,
            tc.tile_pool(name="wpool", bufs=2) as wpool,
            tc.tile_pool(name="abpool", bufs=2) as abpool,
            tc.tile_pool(name="spool", bufs=2) as spool,
            tc.tile_pool(name="small", bufs=1) as small,
            tc.tile_pool(name="pspool", bufs=4, space="PSUM") as pspool,
        ):
            acc = small.tile([_PART, bn, nchunk], fp32)

            for l in range(lp):
                # Per-layer LoRA factors: bt[l] is B^T (r x out), a[l] is (r x in)
                bt_t = abpool.tile([r, out_dim], fp32, tag="bt")
                nc.sync.dma_start(out=bt_t[:], in_=bt[l])
                a_t = abpool.tile([r, in_dim], fp32, tag="a")
                nc.sync.dma_start(out=a_t[:], in_=a[l])

                # W_l[o, i] = sum_r B[o,r] A[r,i]; stored as [128, oc, in]
                w = wpool.tile([_PART, oc, in_dim], fp32, tag="w")
                for c in range(oc):
                    for h in range(ih):
                        ps = pspool.tile([_PART, n_mm], fp32, tag="ps")
                        nc.tensor.matmul(
                            ps[:],
                            lhsT=bt_t[:, c * _PART:(c + 1) * _PART],
                            rhs=a_t[:, h * n_mm:(h + 1) * n_mm],
                            start=True,
                            stop=True,
                        )
                        nc.scalar.copy(
                            out=w[:, c, h * n_mm:(h + 1) * n_mm], in_=ps[:]
                        )

                for j in range(bn):
                    gt = gpool.tile([_PART, oc, in_dim], fp32, tag="g")
                    g_src = g[l, j].rearrange("(c p) i -> p c i", p=_PART)
                    last = l == lp - 1 and j == bn - 1
                    if last:
                        # Split the final block's DMA per STT chunk so the
                        # tail STTs start on partial data.
                        for h in range(nh):
                            nc.sync.dma_start(
                                out=gt[:, h * cw:(h + 1) * cw, :],
                                in_=g_src[:, h * cw:(h + 1) * cw, :],
                            )
                    else:
                        nc.sync.dma_start(out=gt[:], in_=g_src)
                    for h in range(nh):
                        sc = spool.tile([_PART, cw, in_dim], fp32, tag="sc")
                        nc.vector.scalar_tensor_tensor(
                            out=sc[:],
                            in0=gt[:, h * cw:(h + 1) * cw, :],
                            scalar=1.0,
                            in1=w[:, h * cw:(h + 1) * cw, :],
                            op0=mybir.AluOpType.mult,
                            op1=mybir.AluOpType.mult,
                            accum_out=acc[:, j, l * nh + h:l * nh + h + 1],
                        )

            nc.sync.dma_start(out=out[:], in_=acc[:])

    nc.compile()
    return nc


_NC_CACHE = {}


def _get_module():
    if "nc" not in _NC_CACHE:
        _NC_CACHE["nc"] = build_module()
    return _NC_CACHE["nc"]


def make_in_maps(lora_A, lora_B, gradient):
    lora_A = np.asarray(lora_A, dtype=np.float32)
    lora_B = np.asarray(lora_B, dtype=np.float32)
    gradient = np.asarray(gradient, dtype=np.float32)
    in_maps = []
    for c in range(NCORES):
        sl = slice(LP * c, LP * (c + 1))
        in_maps.append({
            "g": np.ascontiguousarray(gradient[sl].reshape(LP, BN, OUT, IN)),
            "bt": np.ascontiguousarray(lora_B[sl].transpose(0, 2, 1)),
            "a": np.ascontiguousarray(lora_A[sl]),
        })
    return in_maps


def kernel(lora_A, lora_B, gradient, _trace=False, _trace_kwargs=None):
    from concourse.bass_utils import run_bass_kernel_spmd

    nc = _get_module()
    in_maps = make_in_maps(lora_A, lora_B, gradient)
    last_exc = None
    for attempt in range(3):
        try:
            res = run_bass_kernel_spmd(
                nc,
                in_maps,
                core_ids=list(range(NCORES)),
                trace=_trace,
                **(_trace_kwargs or {}),
            )
            break
        except Exception as e:  # transient device wedges (NRT_EXEC_UNIT_...)
            last_exc = e
            import time as _time

            _time.sleep(15 * (attempt + 1))
    else:
        raise last_exc
    total = np.zeros(BN, np.float64)
    for m in res.results:
        total += m["out"].astype(np.float64).sum(axis=(0, 2))
    out = total.astype(np.float32).reshape(B, N)
    if _trace:
        return out, res
    return out



# revision 6
# speedup vs baseline: 1.0697x; 1.0697x over previous
"""Trainium2 Bass kernel for the LoRA-update contraction.

Computes out[b,n] = sum_l <B_l @ A_l, gradient[l,b,n]>_F for
  lora_A    [48, 8, 1024]       (L, R, IN)
  lora_B    [48, 1024, 8]       (L, OUT, R)
  gradient  [48, 4, 2, 1024, 1024]  (L, B, N, OUT, IN)

Strategy (memory-bound problem — gradient is 1.6 GB):
  - Shard L across the 8 NeuronCores (6 layers each, ~201 MB of gradient per
    core). Per-core partial outputs are summed on the host.
  - Per core: W_l = B_l @ A_l is computed on the TensorEngine from bf16
    copies of A/B (error ~1e-3 rel, well within tolerance), one layer ahead
    of the gradient stream so layer boundaries never stall the DMA ring.
  - The gradient is stored partition-major ([128, 8192] per (l,b,n) slab,
    rows 8p..8p+7 on partition p) so every DMA descriptor is a contiguous
    16 KB HBM read.  A fused VectorEngine scalar_tensor_tensor accumulates
    acc[p] += sum_f G[p,f]*W[p,f] per 2 MB block in one pass at fp32.
  - B's columns are pre-permuted on the host so the W matmuls produce W
    directly in the same partition-major layout.
  - All A/B factors are staged with one DMA at kernel start; per-layer
    partial accumulators stream out on the scalar-engine DMA ring, keeping
    the sync-engine ring exclusively for the gradient stream.
"""

import numpy as np

L, R, OUT, IN = 48, 8, 1024, 1024
B, N = 4, 2
NCORES = 8
LP = L // NCORES  # layers per core
BN = B * N

_PART = 128
_RPP = OUT // _PART          # gradient rows per partition (8)
_FREE = _RPP * IN            # free dim of one (l,j) slab (8192)
_QH = 2                      # 2 MB half-slabs per (l,j)
_HF = _FREE // _QH           # elements per half-slab per partition (4096)
_NTAIL = 4                   # tail pieces for the final half-slab


def build_module(lp=LP, bn=BN, in_dim=IN, r=R):
    """Build + compile the per-core Bass module (same program on all cores)."""
    import concourse.bacc as bacc
    import concourse.mybir as mybir
    from concourse.tile import TileContext

    fp32 = mybir.dt.float32
    bf16 = mybir.dt.bfloat16
    n_mm = 512                     # matmul moving-dim tile (one PSUM bank)
    ih = in_dim // n_mm

    nc = bacc.Bacc("TRN2", target_bir_lowering=False, debug=False)

    g = nc.dram_tensor("g", [lp, bn, _PART, _FREE], fp32, kind="ExternalInput").ap()
    # ab[l, r, 0:1024] = B^T with columns permuted so matmul chunk c yields
    # W rows 8p+c on partition p; ab[l, r, 1024:2048] = A[l, r, :].
    ab = nc.dram_tensor("ab", [r, lp * 2 * in_dim], bf16, kind="ExternalInput").ap()
    # Per-(partition, chunk, bn) partial sums; the final tiny reduction over
    # partitions/chunks happens on the host.
    out = nc.dram_tensor("out", [_PART, lp * _QH, bn], fp32, kind="ExternalOutput").ap()
    # Partial sums of the last half-slab's tail pieces 1.._NTAIL-1 (all of
    # which belong to (l=lp-1, j=bn-1)).
    out2 = nc.dram_tensor("out2", [_PART, _NTAIL - 1], fp32, kind="ExternalOutput").ap()

    with TileContext(nc) as tc:
        with (
            tc.tile_pool(name="gpool", bufs=5) as gpool,
            tc.tile_pool(name="wpool", bufs=2) as wpool,
            tc.tile_pool(name="spool", bufs=1) as spool,
            tc.tile_pool(name="apool", bufs=2) as apool,
            tc.tile_pool(name="small", bufs=1) as small,
            tc.tile_pool(name="pspool", bufs=4, space="PSUM") as pspool,
        ):
            ab_t = small.tile([r, lp * 2 * in_dim], bf16)
            nc.scalar.dma_start(out=ab_t[:], in_=ab)
            tacc = small.tile([_PART, _NTAIL - 1], fp32)

            def build_w(l):
                # W_l[8p+c, i] on partition p at w[:, c*1024 + i]
                w = wpool.tile([_PART, _FREE], fp32, tag="w")
                base = l * 2 * in_dim
                for c in range(_RPP):
                    for h in range(ih):
                        ps = pspool.tile([_PART, n_mm], fp32, tag="ps")
                        nc.tensor.matmul(
                            ps[:],
                            lhsT=ab_t[:, base + c * _PART:base + (c + 1) * _PART],
                            rhs=ab_t[:, base + in_dim + h * n_mm:
                                     base + in_dim + (h + 1) * n_mm],
                            start=True,
                            stop=True,
                        )
                        nc.scalar.copy(
                            out=w[:, c * in_dim + h * n_mm:
                                  c * in_dim + (h + 1) * n_mm],
                            in_=ps[:],
                        )
                return w

            ws = [build_w(0), build_w(1)]

            for l in range(lp):
                w = ws[l]
                accL = apool.tile([_PART, _QH, bn], fp32, tag="acc")
                for j in range(bn):
                    last = l == lp - 1 and j == bn - 1
                    for q in range(_QH):
                        if last and q == _QH - 1:
                            # Final half-slab: stream in _NTAIL pieces so the
                            # tail STTs start on partial data.
                            npc = _HF // _NTAIL
                            for t in range(_NTAIL):
                                gt = gpool.tile([_PART, npc], fp32, tag="g")
                                nc.sync.dma_start(
                                    out=gt[:],
                                    in_=g[l, j][:, q * _HF + t * npc:
                                                q * _HF + (t + 1) * npc],
                                )
                                sct = spool.tile([_PART, npc], fp32, tag="sct")
                                nc.vector.scalar_tensor_tensor(
                                    out=sct[:],
                                    in0=gt[:],
                                    scalar=1.0,
                                    in1=w[:, q * _HF + t * npc:
                                          q * _HF + (t + 1) * npc],
                                    op0=mybir.AluOpType.mult,
                                    op1=mybir.AluOpType.mult,
                                    accum_out=(accL[:, q, j:j + 1] if t == 0
                                               else tacc[:, t - 1:t]),
                                )
                            nc.scalar.dma_start(out=out2[:], in_=tacc[:])
                        else:
                            gt = gpool.tile([_PART, _HF], fp32, tag="g")
                            nc.sync.dma_start(
                                out=gt[:], in_=g[l, j][:, q * _HF:(q + 1) * _HF]
                            )
                            sc = spool.tile([_PART, _HF], fp32, tag="sc")
                            nc.vector.scalar_tensor_tensor(
                                out=sc[:],
                                in0=gt[:],
                                scalar=1.0,
                                in1=w[:, q * _HF:(q + 1) * _HF],
                                op0=mybir.AluOpType.mult,
                                op1=mybir.AluOpType.mult,
                                accum_out=accL[:, q, j:j + 1],
                            )
                nc.scalar.dma_start(
                    out=out[:, l * _QH:(l + 1) * _QH, :], in_=accL[:]
                )
                if l + 2 < lp:
                    ws.append(build_w(l + 2))

    nc.compile()
    return nc


_NC_CACHE = {}


def _get_module():
    if "nc" not in _NC_CACHE:
        _NC_CACHE["nc"] = build_module()
    return _NC_CACHE["nc"]


def make_in_maps(lora_A, lora_B, gradient):
    import ml_dtypes

    lora_A = np.asarray(lora_A, dtype=np.float32)
    lora_B = np.asarray(lora_B, dtype=np.float32)
    gradient = np.asarray(gradient, dtype=np.float32)
    in_maps = []
    for c in range(NCORES):
        sl = slice(LP * c, LP * (c + 1))
        # btp[l, r, c*128+p] = B[l, 8p+c, r]
        bt = lora_B[sl].transpose(0, 2, 1)
        btp = bt.reshape(LP, R, _PART, _RPP).transpose(0, 1, 3, 2).reshape(
            LP, R, OUT)
        ab = np.concatenate([btp, lora_A[sl]], axis=2)  # [LP, R, 2048]
        ab = ab.transpose(1, 0, 2).reshape(R, LP * 2 * IN)
        in_maps.append({
            "g": np.ascontiguousarray(
                gradient[sl].reshape(LP, BN, _PART, _FREE)),
            "ab": np.ascontiguousarray(ab.astype(ml_dtypes.bfloat16)),
        })
    return in_maps


def kernel(lora_A, lora_B, gradient, _trace=False, _trace_kwargs=None):
    from concourse.bass_utils import run_bass_kernel_spmd

    nc = _get_module()
    in_maps = make_in_maps(lora_A, lora_B, gradient)
    last_exc = None
    for attempt in range(3):
        try:
            res = run_bass_kernel_spmd(
                nc,
                in_maps,
                core_ids=list(range(NCORES)),
                trace=_trace,
                **(_trace_kwargs or {}),
            )
            break
        except Exception as e:  # transient device wedges (NRT_EXEC_UNIT_...)
            last_exc = e
            import time as _time

            _time.sleep(15 * (attempt + 1))
    else:
        raise last_exc
    total = np.zeros(BN, np.float64)
    for m in res.results:
        per_j = m["out"].astype(np.float64).sum(axis=(0, 1))
        per_j[BN - 1] += m["out2"].astype(np.float64).sum()
        total += per_j
    out = total.astype(np.float32).reshape(B, N)
    if _trace:
        return out, res
    return out
